# revision 33
# baseline (speedup 1.0000x reference)
"""Category-specific linear (MoE routing) Trainium2 kernel.

out[s, t, h] = sum_d x[s, t, d] * W[cat_ids[s], d, h] + b[cat_ids[s], h]

Strategy: expert-parallel over 8 NeuronCores. Each core owns 4 of the 32
experts (16 MB of W each, read from HBM exactly once chip-wide). The host
routes samples to the core owning their category, packs their tokens into
128-token tiles (x pre-transposed to [d, token] so tiles DMA straight into
the matmul's lhsT layout), and un-routes the outputs afterwards.

The Bass program is identical on all 8 cores (SPMD): a fixed set of expert
"slots", each slot = one 4 MB weight load + a fixed number of 128-token
matmul tiles. All per-core variation (which expert, which tokens) lives in
the per-core DRAM buffer *contents* the host prepares. Slot sizes are
specialized per call from the actual category histogram, so padding is
minimal (total tiles = max-core lower bound).

Bias is applied on-device by one extra K=1 accumulating matmul per PSUM
tile (lhsT = ones[1,128], rhs = b[cat][1,512]).
"""

import os
import sys

import numpy as np

if "/opt/trn_rl_repo" not in sys.path and os.path.isdir("/opt/trn_rl_repo"):
    sys.path.insert(0, "/opt/trn_rl_repo")

import concourse.mybir as mybir
from concourse import bacc
from concourse.bass_utils import run_bass_kernel_spmd
from concourse.tile import TileContext

P = 128          # SBUF partitions
N_CORES = 8
EXPERTS_PER_CORE = 4
F32 = mybir.dt.float32
F16 = mybir.dt.float16
NP_F16 = np.float16
# output staging dtype (DRAM store); fp16 halves the store traffic at
# ~6e-4 max relative rounding on outputs
OUT_DT = mybir.dt.float16
NP_OUT = np.float16

_program_cache: dict = {}


def _plan(cat_ids: np.ndarray, num_cats: int, tokens_per_sample: int):
    """Assign experts to cores (LPT, exactly EXPERTS_PER_CORE bins) and derive
    a uniform slot structure: slot_sizes[j] = token-tile capacity of slot j,
    identical on every core."""
    counts = np.bincount(cat_ids, minlength=num_cats)
    # token-tiles needed per expert
    tiles = [int(np.ceil(c * tokens_per_sample / P)) for c in counts]
    experts = [e for e in range(num_cats) if counts[e] > 0]
    experts.sort(key=lambda e: -tiles[e])

    bins = [{"load": 0, "experts": []} for _ in range(N_CORES)]
    for e in experts:
        cand = [b for b in bins if len(b["experts"]) < EXPERTS_PER_CORE]
        cand.sort(key=lambda b: (b["load"], len(b["experts"])))
        cand[0]["experts"].append(e)
        cand[0]["load"] += tiles[e]

    # sorted tile profile per core, padded to EXPERTS_PER_CORE
    profiles = []
    for b in bins:
        prof = sorted((tiles[e] for e in b["experts"]), reverse=True)
        prof += [0] * (EXPERTS_PER_CORE - len(prof))
        profiles.append(prof)
    slot_sizes = [
        max(profiles[c][j] for c in range(N_CORES)) for j in range(EXPERTS_PER_CORE)
    ]
    # per-core expert order matching slot order (largest first, pad with -1)
    core_experts = []
    for b in bins:
        es = sorted(b["experts"], key=lambda e: -tiles[e])
        es += [-1] * (EXPERTS_PER_CORE - len(es))
        core_experts.append(es)
    return core_experts, slot_sizes


def _build_program(slot_sizes, kt: int, h_dim: int):
    """SPMD Bass program for one core. kt = number of 128-row K tiles
    (input_dim / 128); h_dim = hidden dim (multiple of 512)."""
    tiles_total = sum(slot_sizes)
    n_half = h_dim // 512

    nc = bacc.Bacc(enable_partition_id=False)
    wdram = nc.declare_dram_parameter(
        "wbuf", [EXPERTS_PER_CORE, P, kt, h_dim], F16, isOutput=False
    )
    bdram = nc.declare_dram_parameter(
        "bbuf", [EXPERTS_PER_CORE, h_dim], F16, isOutput=False
    )
    xdram = nc.declare_dram_parameter(
        "xtbuf", [tiles_total, P, kt, P], F16, isOutput=False
    )
    odram = nc.declare_dram_parameter(
        "outbuf", [tiles_total, P, h_dim], OUT_DT, isOutput=True
    )

    n_slots = sum(1 for s in slot_sizes if s > 0)
    with TileContext(nc) as tc:
        with (
            tc.tile_pool(name="wp", bufs=n_slots) as wp,
            tc.tile_pool(name="bp", bufs=n_slots) as bp,
            tc.tile_pool(name="bsp", bufs=2) as bsp,
            tc.tile_pool(name="xp", bufs=n_slots) as xp,
            tc.tile_pool(name="op", bufs=6) as op,
            tc.tile_pool(name="cp", bufs=1) as cp,
            tc.tile_pool(name="pp", bufs=7, space="PSUM") as pp,
            tc.tile_pool(name="wu", bufs=1, space="PSUM") as wu,
        ):
            ones = cp.tile([1, P], F16)
            nc.vector.memset(ones[:], 1.0)
            ones_r = cp.tile([1, 512], F16)
            nc.vector.memset(ones_r[:], 1.0)
            # Warm the PE's HAM clock gate (~3.4us of sustained matmuls ->
            # 2.4 GHz) with dependency-free K=1 matmuls while the first
            # weight/activation DMAs are still in flight.
            wups = wu.tile([P, 512], F32, tag="wu")
            for i in range(16):
                nc.tensor.matmul(
                    wups[:],
                    lhsT=ones[:],
                    rhs=ones_r[:],
                    start=True,
                    stop=True,
                    skip_group_check=True,
                )

            # All W (8 MB fp16) and x (2.5 MB) stay SBUF-resident: load-side
            # tiles are written exactly once, so every load DMA issues with
            # no WAR/WAW waits and the two HWDGE rings stream back-to-back.
            # W rides the ACT ring, x/out the SP ring. Slot 0 is chunked
            # finely so the first matmuls start ASAP; later slots use big
            # (efficient) transfers that prefetch under slot-0 compute.
            base = 0
            slots = []
            for j, sz in enumerate(slot_sizes):
                if sz == 0:
                    continue
                bt = bp.tile([1, h_dim], F16, tag="b")
                nc.scalar.dma_start(out=bt[:], in_=bdram[j : j + 1, :])
                xs = xp.tile([P, sz, kt, P], F16, tag="x")
                xsrc = xdram[base : base + sz].rearrange("s p k t -> p s k t")
                wt = wp.tile([P, kt, h_dim], F16, tag="w")
                if not slots:
                    for tt in range(sz):
                        nc.sync.dma_start(out=xs[:, tt], in_=xsrc[:, tt])
                    # chunk by h-half first: the n=0 PSUM group of tile 0
                    # only needs wt[:, :, 0:512]
                    for k0, k1 in ((0, 2), (2, 4), (4, kt)):
                        nc.scalar.dma_start(
                            out=wt[:, k0:k1, 0:512], in_=wdram[j, :, k0:k1, 0:512]
                        )
                    nc.scalar.dma_start(
                        out=wt[:, :, 512:], in_=wdram[j, :, :, 512:]
                    )
                else:
                    nc.sync.dma_start(out=xs[:], in_=xsrc)
                    nc.scalar.dma_start(out=wt[:], in_=wdram[j])
                slots.append((j, sz, base, bt, xs, wt))
                base += sz

            for j, sz, sbase, bt, xs, wt in slots:
                # broadcast b[cat] across partitions once per slot
                # (ones[1,128].T @ b[1,512]); the add is fused into PSUM
                # eviction instead of an extra matmul per PSUM tile.
                bias_sb = bsp.tile([P, h_dim], F32, tag="bb")
                for n in range(n_half):
                    psb = pp.tile([P, 512], F32, tag="ps")
                    nc.tensor.matmul(
                        psb[:],
                        lhsT=ones[:],
                        rhs=bt[:, n * 512 : (n + 1) * 512],
                        start=True,
                        stop=True,
                    )
                    nc.vector.tensor_copy(bias_sb[:, n * 512 : (n + 1) * 512], psb[:])
                for tt in range(sz):
                    ot = op.tile([P, h_dim], OUT_DT, tag="o")
                    for n in range(n_half):
                        ps = pp.tile([P, 512], F32, tag="ps")
                        for k in range(kt):
                            nc.tensor.matmul(
                                ps[:],
                                lhsT=xs[:, tt, k, :],
                                rhs=wt[:, k, n * 512 : (n + 1) * 512],
                                start=(k == 0),
                                stop=(k == kt - 1),
                            )
                        nc.vector.tensor_add(
                            ot[:, n * 512 : (n + 1) * 512],
                            ps[:],
                            bias_sb[:, n * 512 : (n + 1) * 512],
                        )
                    nc.sync.dma_start(out=odram[sbase + tt], in_=ot[:])
    # Bacc.compile runs generate_event_semaphores: hardware instructions can
    # carry at most one sync wait; extras become EventSemaphore nops.
    nc.compile()
    return nc


def _prepare(x, cat_ids, W, b):
    """Host-side routing: build per-core DRAM buffers + token maps."""
    B, T, D = x.shape
    num_cats, _, H = W.shape
    kt = D // P

    core_experts, slot_sizes = _plan(cat_ids, num_cats, T)
    tiles_total = sum(slot_sizes)

    x_flat = np.ascontiguousarray(x, dtype=np.float32).reshape(B * T, D)
    sample_ids = [np.nonzero(cat_ids == e)[0] for e in range(num_cats)]

    in_maps = []
    token_maps = []
    for c in range(N_CORES):
        wbuf = np.zeros((EXPERTS_PER_CORE, P, kt, H), NP_F16)
        bbuf = np.zeros((EXPERTS_PER_CORE, H), NP_F16)
        xt = np.zeros((tiles_total, P, kt, P), NP_F16)
        tok_map = np.full(tiles_total * P, -1, np.int64)

        base = 0
        for j, (e, sz) in enumerate(zip(core_experts[c], slot_sizes)):
            if sz == 0:
                continue
            if e >= 0:
                # W[e]: [(k p), h] -> [p, k, h]
                wbuf[j] = W[e].reshape(kt, P, H).transpose(1, 0, 2).astype(NP_F16)
                bbuf[j] = b[e].astype(NP_F16)
                toks = (sample_ids[e][:, None] * T + np.arange(T)[None, :]).ravel()
                n = len(toks)
                cap = sz * P
                assert n <= cap, (c, j, e, n, cap)
                gathered = np.zeros((cap, D), NP_F16)
                gathered[:n] = x_flat[toks]
                # [tile, t, (k p)] -> [tile, p, k, t]
                xt[base : base + sz] = gathered.reshape(sz, P, kt, P).transpose(
                    0, 3, 2, 1
                )
                tok_map[base * P : base * P + n] = toks
            base += sz

        in_maps.append({"wbuf": wbuf, "bbuf": bbuf, "xtbuf": xt})
        token_maps.append(tok_map)

    return in_maps, token_maps, slot_sizes, kt, H


def run(x, cat_ids, W, b, trace=False, **spmd_kwargs):
    x = np.asarray(x, dtype=np.float32)
    cat_np = np.asarray(cat_ids).astype(np.int64)
    W = np.asarray(W, dtype=np.float32)
    b = np.asarray(b, dtype=np.float32)
    B, T, D = x.shape
    H = W.shape[2]

    in_maps, token_maps, slot_sizes, kt, _ = _prepare(x, cat_np, W, b)

    key = (tuple(slot_sizes), kt, H)
    nc = _program_cache.get(key)
    if nc is None:
        nc = _build_program(slot_sizes, kt, H)
        _program_cache[key] = nc

    res = run_bass_kernel_spmd(
        nc, in_maps, list(range(N_CORES)), trace=trace, **spmd_kwargs
    )

    out_flat = np.empty((B * T, H), np.float32)
    filled = np.zeros(B * T, bool)
    for c in range(N_CORES):
        o = res.results[c]["outbuf"].reshape(-1, H).astype(np.float32)
        m = token_maps[c] >= 0
        out_flat[token_maps[c][m]] = o[m]
        filled[token_maps[c][m]] = True
    assert filled.all()
    return out_flat.reshape(B, T, H), res


def kernel(x, cat_ids, W, b):
    out, _ = run(x, cat_ids, W, b, trace=False)
    return out
